# revision 23
# baseline (speedup 1.0000x reference)
"""Differentiable ECE (soft histogram binning) on 8 trn2 NeuronCores.

Math: reference computes, for 10 bin centers c_b = 0.05 + 0.1*b,
    w_b(p) = exp(-(p-c_b)^2 / 0.02)           (1/0.02 = 50)
    S_b = sum_n w_b;  C_b = sum_n w_b p_n;  A_b = sum_n w_b l_n
    ECE = sum_b (S_b/(S_b+eps)) * |C_b - A_b| / (S_b+eps)

Key reductions exploited by the kernel:
  * Only D_b = C_b - A_b is needed, never C_b and A_b separately, so a single
    weighted chain over d = p - l covers both moments.
  * w_b = w_0 * r^b * Q_b with r = exp(10p) (host-precomputed bf16) and
    scalar Q_b = exp(-b(b+1)/2), so each further bin costs one bf16
    tensor_tensor multiply (DVE 2x mode) instead of an exp.
  * Derivative_Erf(sqrt(50)*(p-c_b)) = (2/sqrt(pi)) exp(-50(p-c_b)^2) lets
    the scalar engine produce any S_b in ONE activation pass with fused
    per-partition accumulation.  Bins are split between the ACT path and a
    short DVE u-chain to balance the two engines.
  * All chain tiles are reduced by the tensor engine: one-hot bf16
    stationary matrices accumulate column sums of every tile into a
    [13, 512] PSUM region (start=False accumulation across all chunks).

Per core per chunk: 7 ACT passes, 13 DVE tensor_tensor passes, 13*(F/512)
matmuls.  Host finishes the tiny partial-sum tensors in float64.

Sharding: data-parallel, flattened element axis split evenly across 8 cores.
"""

import sys

sys.path.insert(0, "/opt/trn_rl_repo")

import math
from contextlib import ExitStack

import ml_dtypes
import numpy as np

import concourse.bass as bass
import concourse.tile as tile
from concourse import bacc, mybir
from concourse.bass_utils import run_bass_kernel_spmd

N_CORES = 8
P_DIM = 128
ROWS, COLS = 2048, 8192
F_TOT = ROWS * COLS // N_CORES // P_DIM  # 16384 free elems per partition per core
CHUNKS = [512, 1024, 2560, 4096, 4096, 4096]  # ramp-up schedule, sums to F_TOT
NCH = len(CHUNKS)
NB = 10                                  # bins
K_CHAIN = 3                              # S_1..S_3 via DVE u-chain, rest via ACT
NQ = NB + K_CHAIN                        # 13 matmul-reduced quantities
N_ACT_BINS = NB - K_CHAIN                # 7 S bins on ACT (b=0 and b=4..9)
J = 512                                  # matmul moving free dim
EPS = 1e-8
SQ50 = math.sqrt(50.0)

_cache = {}


def _build():
    nc = bacc.Bacc("TRN2", target_bir_lowering=False, debug=False)
    f32, bf16 = mybir.dt.float32, mybir.dt.bfloat16
    Act = mybir.ActivationFunctionType

    # Register const APs for the activation biases -sqrt(50)*c_b (activation()
    # requires non-Copy bias as a const AP, same mechanism as Bass.__init__).
    centers = [0.05 + 0.1 * b for b in range(NB)]
    biases = [float(np.float32(-SQ50 * c)) for c in centers]
    for i, v in enumerate(biases):
        t = nc.alloc_sbuf_tensor(f"const-bias-{i}", [128, 1], f32)
        nc.gpsimd.memset(t.ap(), v)
        nc.const_aps.aps[(f32, v)] = t.ap()
    nc.all_engine_barrier()

    p32 = nc.dram_tensor("p32", [P_DIM, F_TOT], f32, kind="ExternalInput").ap()
    db = nc.dram_tensor("db", [P_DIM, F_TOT], bf16, kind="ExternalInput").ap()
    rb = nc.dram_tensor("rb", [P_DIM, F_TOT], bf16, kind="ExternalInput").ap()
    emat = nc.dram_tensor("emat", [P_DIM, NQ * NQ], bf16, kind="ExternalInput").ap()
    acc = nc.dram_tensor("acc", [NQ, J], f32, kind="ExternalOutput").ap()
    accs = nc.dram_tensor(
        "accs", [P_DIM, N_ACT_BINS * NCH], f32, kind="ExternalOutput"
    ).ap()

    n_mm_total = NQ * sum(f // J for f in CHUNKS)

    with tile.TileContext(nc) as tc, ExitStack() as ctx:
        pool_c = ctx.enter_context(tc.tile_pool(name="const", bufs=1))
        pool_p = ctx.enter_context(tc.tile_pool(name="p", bufs=3))
        pool_b = ctx.enter_context(tc.tile_pool(name="b", bufs=3))
        pool_w = ctx.enter_context(tc.tile_pool(name="w", bufs=3))
        pool_ps = ctx.enter_context(tc.tile_pool(name="ps", bufs=1, space="PSUM"))

        em = pool_c.tile([P_DIM, NQ * NQ], bf16)
        nc.gpsimd.dma_start(em[:], emat[:])
        ps = pool_ps.tile([NQ, J], f32)
        accs_t = pool_c.tile([P_DIM, N_ACT_BINS * NCH], f32)
        junk = pool_c.tile([P_DIM, max(CHUNKS)], bf16)

        mm_count = [0]

        def reduce_into(row, t, fsz):
            for j0 in range(0, fsz, J):
                i = mm_count[0]
                nc.tensor.matmul(
                    ps[:, :],
                    em[:, row * NQ : (row + 1) * NQ],
                    t[:, j0 : j0 + J],
                    start=(i == 0),
                    stop=(i == n_mm_total - 1),
                )
                mm_count[0] += 1

        off = 0
        for ci, F in enumerate(CHUNKS):
            sl = slice(off, off + F)
            off += F
            pf = pool_p.tile([P_DIM, F], f32, tag="pf")
            nc.sync.dma_start(pf[:], p32[:, sl])
            dbt = pool_b.tile([P_DIM, F], bf16, tag="db")
            nc.sync.dma_start(dbt[:], db[:, sl])
            rbt = pool_b.tile([P_DIM, F], bf16, tag="rb")
            nc.sync.dma_start(rbt[:], rb[:, sl])

            # u0 = (2/sqrt(pi)) exp(-50 (p-0.05)^2), S'_0 accumulated
            u0 = pool_w.tile([P_DIM, F], bf16, tag="u0")
            nc.scalar.activation(
                u0[:], pf[:], Act.Derivative_Erf,
                bias=biases[0], scale=SQ50,
                accum_out=accs_t[:, ci * N_ACT_BINS : ci * N_ACT_BINS + 1],
            )

            # S'_b for b=K_CHAIN+1..9: accumulate-only Derivative_Erf passes
            # (emitted right after u0 so ACT never trails the chunk)
            for b in range(K_CHAIN + 1, NB):
                slot = ci * N_ACT_BINS + (b - K_CHAIN)
                nc.scalar.activation(
                    junk[:, :F], pf[:], Act.Derivative_Erf,
                    bias=biases[b], scale=SQ50,
                    accum_out=accs_t[:, slot : slot + 1],
                )

            # DVE chains: ud_b = u0 * d * r^b (rows 0..9),
            #             u_b = u0 * r^b for b=1..K_CHAIN (rows 10..12)
            ud = pool_w.tile([P_DIM, F], bf16, tag="ud")
            nc.vector.tensor_mul(ud[:], u0[:], dbt[:])
            reduce_into(0, ud, F)
            u = u0
            for b in range(1, K_CHAIN + 1):
                u2 = pool_w.tile([P_DIM, F], bf16, tag="u")
                nc.vector.tensor_mul(u2[:], u[:], rbt[:])
                u = u2
                reduce_into(NB + b - 1, u, F)
                ud2 = pool_w.tile([P_DIM, F], bf16, tag="ud")
                nc.vector.tensor_mul(ud2[:], ud[:], rbt[:])
                ud = ud2
                reduce_into(b, ud, F)
            for b in range(K_CHAIN + 1, NB):
                ud2 = pool_w.tile([P_DIM, F], bf16, tag="ud")
                nc.vector.tensor_mul(ud2[:], ud[:], rbt[:])
                ud = ud2
                reduce_into(b, ud, F)

        outsb = pool_c.tile([NQ, J], f32)
        nc.vector.tensor_copy(outsb[:], ps[:])
        nc.gpsimd.dma_start(acc[:], outsb[:])
        nc.gpsimd.dma_start(accs[:], accs_t[:])

    nc.finalize()
    return nc


def _get_nc():
    if "nc" not in _cache:
        _cache["nc"] = _build()
    return _cache["nc"]


def _prep_in_maps(probs, labels):
    p = np.ascontiguousarray(np.asarray(probs, dtype=np.float32)).reshape(
        N_CORES, P_DIM, F_TOT
    )
    lab = np.ascontiguousarray(np.asarray(labels)).reshape(N_CORES, P_DIM, F_TOT)
    dbf = (p - lab.astype(np.float32)).astype(ml_dtypes.bfloat16)
    rbf = np.exp(10.0 * p).astype(ml_dtypes.bfloat16)
    em = np.zeros((NQ, NQ), dtype=ml_dtypes.bfloat16)
    np.fill_diagonal(em, 1.0)
    em = np.tile(em.reshape(1, NQ * NQ), (P_DIM, 1))
    return [
        {"p32": p[i], "db": dbf[i], "rb": rbf[i], "emat": em}
        for i in range(N_CORES)
    ]


def _finish(results):

    rows = np.zeros(NQ, dtype=np.float64)
    s_act = np.zeros(N_ACT_BINS, dtype=np.float64)
    for i in range(N_CORES):
        rows += results[i]["acc"].astype(np.float64).sum(axis=1)
        a = results[i]["accs"].astype(np.float64).reshape(P_DIM, NCH, N_ACT_BINS)
        s_act += a.sum(axis=(0, 1))

    b = np.arange(NB, dtype=np.float64)
    Q = np.exp(-0.5 * (b * b + b))
    HSP = math.sqrt(math.pi) / 2.0

    S = np.zeros(NB)
    S[0] = s_act[0] * HSP
    for bb in range(1, K_CHAIN + 1):
        S[bb] = rows[NB + bb - 1] * Q[bb] * HSP
    for bb in range(K_CHAIN + 1, NB):
        S[bb] = s_act[bb - K_CHAIN] * HSP
    D = rows[0:NB] * Q * HSP

    denom = S + EPS
    ece = ((S / denom) * np.abs(D) / denom).sum()
    return np.float32(ece)


def kernel(probs, labels):
    nc = _get_nc()
    in_maps = _prep_in_maps(probs, labels)
    res = run_bass_kernel_spmd(nc, in_maps, list(range(N_CORES)))
    return _finish(res.results)


# revision 25
# speedup vs baseline: 1.0140x; 1.0140x over previous
"""Differentiable ECE (soft histogram binning) on 8 trn2 NeuronCores.

Math: reference computes, for 10 bin centers c_b = 0.05 + 0.1*b,
    w_b(p) = exp(-(p-c_b)^2 / 0.02)           (1/0.02 = 50)
    S_b = sum_n w_b;  C_b = sum_n w_b p_n;  A_b = sum_n w_b l_n
    ECE = sum_b (S_b/(S_b+eps)) * |C_b - A_b| / (S_b+eps)

Key reductions exploited by the kernel:
  * Only D_b = C_b - A_b is needed, never C_b and A_b separately, so a single
    weighted chain over d = p - l covers both moments.
  * w_b = w_0 * r^b * Q_b with r = exp(10p) (host-precomputed bf16) and
    scalar Q_b = exp(-b(b+1)/2), so each further bin costs one bf16
    tensor_tensor multiply (DVE 2x mode) instead of an exp.
  * Derivative_Erf(sqrt(50)*(p-c_b)) = (2/sqrt(pi)) exp(-50(p-c_b)^2) lets
    the scalar engine produce any S_b in ONE activation pass with fused
    per-partition accumulation.  Bins are split between the ACT path and a
    short DVE u-chain to balance the two engines.
  * All chain tiles are reduced by the tensor engine: one-hot bf16
    stationary matrices accumulate column sums of every tile into a
    [13, 512] PSUM region (start=False accumulation across all chunks).

Per core per chunk: 7 ACT passes, 13 DVE tensor_tensor passes, 13*(F/512)
matmuls.  Host finishes the tiny partial-sum tensors in float64.

Sharding: data-parallel, flattened element axis split evenly across 8 cores.
"""

import sys

sys.path.insert(0, "/opt/trn_rl_repo")

import math
from contextlib import ExitStack

import ml_dtypes
import numpy as np

import concourse.bass as bass
import concourse.tile as tile
from concourse import bacc, mybir
from concourse.bass_utils import run_bass_kernel_spmd

N_CORES = 8
P_DIM = 128
ROWS, COLS = 2048, 8192
F_TOT = ROWS * COLS // N_CORES // P_DIM  # 16384 free elems per partition per core
CHUNKS = [1024, 3072, 4096, 4096, 3072, 1024]  # ramp up AND taper down, sums to F_TOT
NCH = len(CHUNKS)
NB = 10                                  # bins
K_CHAIN = 3                              # S_1..S_3 via DVE u-chain, rest via ACT
NQ = NB + K_CHAIN                        # 13 matmul-reduced quantities
N_ACT_BINS = NB - K_CHAIN                # 7 S bins on ACT (b=0 and b=4..9)
J = 512                                  # matmul moving free dim
EPS = 1e-8
SQ50 = math.sqrt(50.0)

_cache = {}


def _build():
    nc = bacc.Bacc("TRN2", target_bir_lowering=False, debug=False)
    f32, bf16 = mybir.dt.float32, mybir.dt.bfloat16
    Act = mybir.ActivationFunctionType

    # Register const APs for the activation biases -sqrt(50)*c_b (activation()
    # requires non-Copy bias as a const AP, same mechanism as Bass.__init__).
    centers = [0.05 + 0.1 * b for b in range(NB)]
    biases = [float(np.float32(-SQ50 * c)) for c in centers]
    for i, v in enumerate(biases):
        t = nc.alloc_sbuf_tensor(f"const-bias-{i}", [128, 1], f32)
        nc.gpsimd.memset(t.ap(), v)
        nc.const_aps.aps[(f32, v)] = t.ap()
    nc.all_engine_barrier()

    p32 = nc.dram_tensor("p32", [P_DIM, F_TOT], f32, kind="ExternalInput").ap()
    db = nc.dram_tensor("db", [P_DIM, F_TOT], bf16, kind="ExternalInput").ap()
    rb = nc.dram_tensor("rb", [P_DIM, F_TOT], bf16, kind="ExternalInput").ap()
    emat = nc.dram_tensor("emat", [P_DIM, NQ * NQ], bf16, kind="ExternalInput").ap()
    acc = nc.dram_tensor("acc", [NQ, J], f32, kind="ExternalOutput").ap()
    accs = nc.dram_tensor(
        "accs", [P_DIM, N_ACT_BINS * NCH], f32, kind="ExternalOutput"
    ).ap()

    n_mm_total = NQ * sum(f // J for f in CHUNKS)

    with tile.TileContext(nc) as tc, ExitStack() as ctx:
        pool_c = ctx.enter_context(tc.tile_pool(name="const", bufs=1))
        pool_p = ctx.enter_context(tc.tile_pool(name="p", bufs=2))
        pool_b = ctx.enter_context(tc.tile_pool(name="b", bufs=2))
        pool_w = ctx.enter_context(tc.tile_pool(name="w", bufs=3))
        pool_ps = ctx.enter_context(tc.tile_pool(name="ps", bufs=1, space="PSUM"))

        em = pool_c.tile([P_DIM, NQ * NQ], bf16)
        nc.gpsimd.dma_start(em[:], emat[:])
        ps = pool_ps.tile([NQ, J], f32)
        accs_t = pool_c.tile([P_DIM, N_ACT_BINS * NCH], f32)
        junk = pool_c.tile([P_DIM, max(CHUNKS)], bf16)

        mm_count = [0]

        def reduce_into(row, t, fsz):
            for j0 in range(0, fsz, J):
                i = mm_count[0]
                nc.tensor.matmul(
                    ps[:, :],
                    em[:, row * NQ : (row + 1) * NQ],
                    t[:, j0 : j0 + J],
                    start=(i == 0),
                    stop=(i == n_mm_total - 1),
                )
                mm_count[0] += 1

        off = 0
        for ci, F in enumerate(CHUNKS):
            sl = slice(off, off + F)
            off += F
            pf = pool_p.tile([P_DIM, F], f32, tag="pf")
            nc.sync.dma_start(pf[:], p32[:, sl])
            dbt = pool_b.tile([P_DIM, F], bf16, tag="db")
            nc.sync.dma_start(dbt[:], db[:, sl])
            rbt = pool_b.tile([P_DIM, F], bf16, tag="rb")
            nc.sync.dma_start(rbt[:], rb[:, sl])

            # u0 = (2/sqrt(pi)) exp(-50 (p-0.05)^2), S'_0 accumulated
            u0 = pool_w.tile([P_DIM, F], bf16, tag="u0")
            nc.scalar.activation(
                u0[:], pf[:], Act.Derivative_Erf,
                bias=biases[0], scale=SQ50,
                accum_out=accs_t[:, ci * N_ACT_BINS : ci * N_ACT_BINS + 1],
            )

            # S'_b for b=K_CHAIN+1..9: accumulate-only Derivative_Erf passes
            # (emitted right after u0 so ACT never trails the chunk)
            for b in range(K_CHAIN + 1, NB):
                slot = ci * N_ACT_BINS + (b - K_CHAIN)
                nc.scalar.activation(
                    junk[:, :F], pf[:], Act.Derivative_Erf,
                    bias=biases[b], scale=SQ50,
                    accum_out=accs_t[:, slot : slot + 1],
                )

            # DVE chains: ud_b = u0 * d * r^b (rows 0..9),
            #             u_b = u0 * r^b for b=1..K_CHAIN (rows 10..12)
            ud = pool_w.tile([P_DIM, F], bf16, tag="ud")
            nc.vector.tensor_mul(ud[:], u0[:], dbt[:])
            reduce_into(0, ud, F)
            u = u0
            for b in range(1, K_CHAIN + 1):
                u2 = pool_w.tile([P_DIM, F], bf16, tag="u")
                nc.vector.tensor_mul(u2[:], u[:], rbt[:])
                u = u2
                reduce_into(NB + b - 1, u, F)
                ud2 = pool_w.tile([P_DIM, F], bf16, tag="ud")
                nc.vector.tensor_mul(ud2[:], ud[:], rbt[:])
                ud = ud2
                reduce_into(b, ud, F)
            for b in range(K_CHAIN + 1, NB):
                ud2 = pool_w.tile([P_DIM, F], bf16, tag="ud")
                nc.vector.tensor_mul(ud2[:], ud[:], rbt[:])
                ud = ud2
                reduce_into(b, ud, F)

        outsb = pool_c.tile([NQ, J], f32)
        nc.vector.tensor_copy(outsb[:], ps[:])
        nc.gpsimd.dma_start(acc[:], outsb[:])
        nc.gpsimd.dma_start(accs[:], accs_t[:])

    nc.finalize()
    return nc


def _get_nc():
    if "nc" not in _cache:
        _cache["nc"] = _build()
    return _cache["nc"]


def _prep_in_maps(probs, labels):
    p = np.ascontiguousarray(np.asarray(probs, dtype=np.float32)).reshape(
        N_CORES, P_DIM, F_TOT
    )
    lab = np.ascontiguousarray(np.asarray(labels)).reshape(N_CORES, P_DIM, F_TOT)
    dbf = (p - lab.astype(np.float32)).astype(ml_dtypes.bfloat16)
    rbf = np.exp(10.0 * p).astype(ml_dtypes.bfloat16)
    em = np.zeros((NQ, NQ), dtype=ml_dtypes.bfloat16)
    np.fill_diagonal(em, 1.0)
    em = np.tile(em.reshape(1, NQ * NQ), (P_DIM, 1))
    return [
        {"p32": p[i], "db": dbf[i], "rb": rbf[i], "emat": em}
        for i in range(N_CORES)
    ]


def _finish(results):

    rows = np.zeros(NQ, dtype=np.float64)
    s_act = np.zeros(N_ACT_BINS, dtype=np.float64)
    for i in range(N_CORES):
        rows += results[i]["acc"].astype(np.float64).sum(axis=1)
        a = results[i]["accs"].astype(np.float64).reshape(P_DIM, NCH, N_ACT_BINS)
        s_act += a.sum(axis=(0, 1))

    b = np.arange(NB, dtype=np.float64)
    Q = np.exp(-0.5 * (b * b + b))
    HSP = math.sqrt(math.pi) / 2.0

    S = np.zeros(NB)
    S[0] = s_act[0] * HSP
    for bb in range(1, K_CHAIN + 1):
        S[bb] = rows[NB + bb - 1] * Q[bb] * HSP
    for bb in range(K_CHAIN + 1, NB):
        S[bb] = s_act[bb - K_CHAIN] * HSP
    D = rows[0:NB] * Q * HSP

    denom = S + EPS
    ece = ((S / denom) * np.abs(D) / denom).sum()
    return np.float32(ece)


def kernel(probs, labels):
    nc = _get_nc()
    in_maps = _prep_in_maps(probs, labels)
    res = run_bass_kernel_spmd(nc, in_maps, list(range(N_CORES)))
    return _finish(res.results)
